# revision 7
# baseline (speedup 1.0000x reference)
"""Bahdanau attention TRN2 Bass kernel.

Shapes (hardcoded): B=32, S=2048, ENC=DEC=1024, fp32.
Sharding: data-parallel over batch B across 8 NeuronCores (4 batches/core);
W_h / W_s / v replicated.

Per-core algorithm (b in 0..3):
  enc_T[d, s]  = sum_e W_h[d, e] * encoder[b, s, e]        (PE, fp32r)
                 encoder tiles transposed on-chip via PE transpose-mode
  t[d, s]      = tanh(enc_T[d, s] + dec[b, d])             (ACT, fused bias)
  energy[s]    = sum_d v[d] * t[d, s]                      (PE, M=1 matmul)
  w[s]         = exp(energy - max) * mask                  (ACT + DVE fused reduce)
  attn[s]      = w[s] / sum(w)                             (DVE)
  context[e]   = (sum_s w[s] * encoder[b, s, e]) / sum(w)  (PE, M=1 matmul)
"""

import numpy as np

import concourse.bacc as bacc
import concourse.bass as bass
import concourse.mybir as mybir
import concourse.tile as tile
from concourse.bass_utils import run_bass_kernel_spmd
from concourse.masks import make_identity

B, S, E, D = 32, 2048, 1024, 1024
NCORES = 8
BLOC = B // NCORES  # 4 batches per core
P = 128
SCH = 512           # s-chunk processed per inner iteration
NCHUNK = S // SCH   # 4
ET = E // P         # 8 e-tiles (contraction tiles for the big matmul)
DT = D // P         # 8 d-tiles
ST = S // P         # 16 s-tiles
F32 = mybir.dt.float32
F32R = mybir.dt.float32r
I32 = mybir.dt.int32
AF = mybir.ActivationFunctionType
ALU = mybir.AluOpType


def _r(ap):
    """View an fp32 AP as float32r for full-rate PE matmuls."""
    return ap.bitcast(F32R)


def build_kernel():
    nc = bacc.Bacc("TRN2", target_bir_lowering=False, debug=False)

    enc_d = nc.dram_tensor("encoder_outputs", [BLOC, S, E], F32, kind="ExternalInput")
    h_d = nc.dram_tensor("decoder_hidden", [BLOC, D], F32, kind="ExternalInput")
    mask_d = nc.dram_tensor("mask", [BLOC, S], I32, kind="ExternalInput")
    wh_d = nc.dram_tensor("W_h", [D, E], F32, kind="ExternalInput")
    ws_d = nc.dram_tensor("W_s", [D, D], F32, kind="ExternalInput")
    v_d = nc.dram_tensor("v", [D], F32, kind="ExternalInput")
    ctx_d = nc.dram_tensor("context", [BLOC, E], F32, kind="ExternalOutput")
    attn_d = nc.dram_tensor("attn_weights", [BLOC, S], F32, kind="ExternalOutput")

    with tile.TileContext(nc) as tc:
        with (
            tc.tile_pool(name="singles", bufs=1) as sb,
            tc.tile_pool(name="psum", bufs=1, space="PSUM") as psum,
            tc.tile_pool(name="dramp", bufs=1, space="DRAM") as dram,
        ):
            ident = sb.tile([P, P], F32, name="ident")
            make_identity(nc, ident)

            # v striped: v_sb[p, o] = v[o*P + p]
            v_sb = sb.tile([P, DT], F32R, name="v_sb")
            nc.sync.dma_start(
                out=v_sb,
                in_=v_d[:].rearrange("(o p) -> p o", p=P).bitcast(F32R))

            # whT[p, e_t, d] = W_h[d, e_t*P + p]  -- persistent, 32KB/part
            whT = sb.tile([P, ET, D], F32R, name="whT")
            # hT[p, k, b] = h[b, k*P + p]
            hT = sb.tile([P, DT, BLOC], F32, name="hT")
            for o in range(DT):
                nc.sync.dma_start(
                    out=hT[:, o, :],
                    in_=h_d[:, o * P:(o + 1) * P].rearrange("b p -> p b"))
            # decT[p, o, b] = dec[b, o*P + p]
            decT = sb.tile([P, DT, BLOC], F32, name="decT")

            # ---------- setup (transient pool, released afterward) ----------
            with tc.tile_pool(name="setup", bufs=1) as sp:
                wh_nat = wh_d[:].rearrange("(o p) e -> p o e", p=P)
                for o in range(DT):
                    wnat = sp.tile([P, E], F32, tag="wnat", bufs=3, name="wnat")
                    nc.sync.dma_start(out=wnat, in_=wh_nat[:, o, :])
                    for e_t in range(ET):
                        pt = psum.tile([P, P], F32, tag="pt", bufs=2, name="ptw")
                        nc.tensor.transpose(pt, wnat[:, e_t * P:(e_t + 1) * P],
                                            ident)
                        dst = whT[:, e_t, o * P:(o + 1) * P]
                        if e_t % 2 == 0:
                            nc.vector.tensor_copy(out=dst, in_=pt)
                        else:
                            nc.scalar.copy(out=dst, in_=pt)

                # wsT[p, d_t, e] = W_s[e, d_t*P + p] -- setup only
                wsT = sp.tile([P, DT, D], F32, name="wsT")
                ws_nat = ws_d[:].rearrange("(o p) d -> p o d", p=P)
                for o in range(DT):
                    wnat = sp.tile([P, D], F32, tag="wnat", bufs=3, name="wnat")
                    nc.sync.dma_start(out=wnat, in_=ws_nat[:, o, :])
                    for d_t in range(DT):
                        pt = psum.tile([P, P], F32, tag="pt", bufs=2, name="ptw")
                        nc.tensor.transpose(pt, wnat[:, d_t * P:(d_t + 1) * P],
                                            ident)
                        dst = wsT[:, d_t, o * P:(o + 1) * P]
                        if d_t % 2 == 0:
                            nc.vector.tensor_copy(out=dst, in_=pt)
                        else:
                            nc.scalar.copy(out=dst, in_=pt)

                # dec = h @ W_s^T (striped into decT)
                for o in range(DT):
                    pdec = psum.tile([P, BLOC], F32, tag="pm", bufs=2, name="pdec")
                    for k in range(DT):
                        nc.tensor.matmul(
                            pdec,
                            lhsT=wsT[:, k, o * P:(o + 1) * P],
                            rhs=hT[:, k, :],
                            start=(k == 0),
                            stop=(k == DT - 1),
                        )
                    nc.vector.tensor_copy(out=decT[:, o, :], in_=pdec)

            # ---------- main loop ----------
            with tc.tile_pool(name="work", bufs=1) as work:
                energy_t, rz_t, wcol_t = {}, {}, {}

                def chunks(b):
                    energy = work.tile([1, S], F32, tag="energy", bufs=2,
                                       name=f"energy{b}")
                    energy_t[b] = energy
                    for c in range(NCHUNK):
                        nats = []
                        for st in range(SCH // P):
                            nat = work.tile([P, E], F32, tag="nat", bufs=6,
                                            name=f"nat{b}_{c}_{st}")
                            s0 = c * SCH + st * P
                            nc.sync.dma_start(out=nat, in_=enc_d[b, s0:s0 + P, :])
                            nats.append(nat)
                        # transpose chunk into encT[p, e_t, s]
                        encT = work.tile([P, ET, SCH], F32R, tag="encT", bufs=2,
                                         name=f"encT{b}_{c}")
                        for e_t in range(ET):
                            pt = psum.tile([P, SCH], F32, tag="pt", bufs=2,
                                           name="ptc")
                            for st in range(SCH // P):
                                nc.tensor.transpose(
                                    pt[:, st * P:(st + 1) * P],
                                    nats[st][:, e_t * P:(e_t + 1) * P],
                                    ident,
                                )
                            if e_t % 2 == 0:
                                nc.vector.tensor_copy(out=encT[:, e_t, :], in_=pt)
                            else:
                                nc.scalar.copy(out=encT[:, e_t, :], in_=pt)
                        # big matmul + tanh + v-reduction
                        pe_ = psum.tile([1, SCH], F32, tag="pe", bufs=2, name="pex")
                        for o in range(DT):
                            pm = psum.tile([P, SCH], F32, tag="pm", bufs=2,
                                           name="pmx")
                            for e_t in range(ET):
                                nc.tensor.matmul(
                                    pm,
                                    lhsT=whT[:, e_t, o * P:(o + 1) * P],
                                    rhs=encT[:, e_t, :],
                                    start=(e_t == 0),
                                    stop=(e_t == ET - 1),
                                )
                            th = work.tile([P, SCH], F32R, tag="tanh", bufs=3,
                                           name="th")
                            nc.scalar.activation(
                                out=th, in_=pm, func=AF.Tanh,
                                bias=decT[:, o, b:b + 1], scale=1.0,
                            )
                            nc.tensor.matmul(
                                pe_,
                                lhsT=v_sb[:, o:o + 1],
                                rhs=th,
                                start=(o == 0),
                                stop=(o == DT - 1),
                            )
                        nc.vector.tensor_copy(out=energy[:, c * SCH:(c + 1) * SCH],
                                              in_=pe_)

                def softmax(b):
                    energy = energy_t[b]
                    # mask row -> fp32 (loaded lazily to cap SBUF rows)
                    mi = work.tile([1, S], I32, tag="maski", bufs=2, name="mi")
                    nc.sync.dma_start(out=mi, in_=mask_d[b:b + 1, :])
                    mf = work.tile([1, S], F32, tag="maskf", bufs=2, name="mf")
                    nc.vector.tensor_copy(out=mf, in_=mi)

                    negmax = work.tile([1, 1], F32, tag="negmax", bufs=2,
                                       name=f"negmax{b}")
                    nc.vector.tensor_reduce(out=negmax, in_=energy,
                                            axis=mybir.AxisListType.X,
                                            op=ALU.max, negate=True)
                    wm = work.tile([1, S], F32, tag="wm", bufs=2, name=f"wm{b}")
                    nc.scalar.activation(out=wm, in_=energy, func=AF.Exp,
                                         bias=negmax, scale=1.0)
                    # masked unnormalized weights -> reuse the energy tile
                    zsum = work.tile([1, 1], F32, tag="zsum", bufs=2,
                                     name=f"z{b}")
                    nc.vector.tensor_mul(energy, wm, mf)
                    nc.vector.reduce_sum(zsum, energy,
                                         axis=mybir.AxisListType.X)
                    rz = work.tile([1, 1], F32, tag="rz", bufs=2, name=f"rz{b}")
                    nc.vector.reciprocal(rz, zsum)
                    rz_t[b] = rz
                    # attn = masked * rz, into the wm tile
                    nc.vector.tensor_scalar_mul(wm, energy, rz)
                    nc.sync.dma_start(out=attn_d[b:b + 1, :], in_=wm)
                    # unnormalized weights to column form via a DRAM bounce:
                    # wcol[p, t] = w_masked[t*P + p]
                    wrow = dram.tile([1, S], F32, tag="wrow", bufs=2,
                                     name=f"wrow{b}")
                    nc.sync.dma_start(out=wrow, in_=energy)
                    wcol = work.tile([P, ST], F32R, tag="wcol", bufs=2,
                                     name=f"wcol{b}")
                    nc.sync.dma_start(
                        out=wcol,
                        in_=wrow[:].rearrange("o (t p) -> (o p) t", p=P)
                        .bitcast(F32R))
                    wcol_t[b] = wcol

                def context(b):
                    wcol, rz = wcol_t[b], rz_t[b]
                    pc0 = psum.tile([1, SCH], F32, tag="pc", bufs=2, name="pc0")
                    pc1 = psum.tile([1, SCH], F32, tag="pc", bufs=2, name="pc1")
                    for t in range(ST):
                        rl = work.tile([P, E], F32R, tag="rl", bufs=5,
                                       name=f"rl{b}_{t}")
                        nc.sync.dma_start(
                            out=rl,
                            in_=enc_d[b, t * P:(t + 1) * P, :].bitcast(F32R))
                        wc = wcol[:, t:t + 1]
                        nc.tensor.matmul(pc0, lhsT=wc, rhs=rl[:, 0:SCH],
                                         start=(t == 0), stop=(t == ST - 1))
                        nc.tensor.matmul(pc1, lhsT=wc, rhs=rl[:, SCH:E],
                                         start=(t == 0), stop=(t == ST - 1))
                    ctx_sb = work.tile([1, E], F32, tag="ctx", bufs=2,
                                       name=f"ctx{b}")
                    nc.scalar.activation(out=ctx_sb[:, 0:SCH], in_=pc0,
                                         func=AF.Copy, bias=0.0, scale=rz)
                    nc.scalar.activation(out=ctx_sb[:, SCH:E], in_=pc1,
                                         func=AF.Copy, bias=0.0, scale=rz)
                    nc.sync.dma_start(out=ctx_d[b:b + 1, :], in_=ctx_sb)

                # emission order: defer batch b's context past batch b+1's
                # heavy PE phase so the PE never waits on the softmax chain.
                for b in range(BLOC):
                    chunks(b)
                    softmax(b)
                    if b > 0:
                        context(b - 1)
                context(BLOC - 1)

    nc.compile()
    return nc


_NC = None


def _get_nc():
    global _NC
    if _NC is None:
        _NC = build_kernel()
    return _NC


def _make_in_maps(inputs):
    in_maps = []
    for i in range(NCORES):
        lo, hi = i * BLOC, (i + 1) * BLOC
        in_maps.append({
            "encoder_outputs": np.ascontiguousarray(
                inputs["encoder_outputs"][lo:hi], dtype=np.float32),
            "decoder_hidden": np.ascontiguousarray(
                inputs["decoder_hidden"][lo:hi], dtype=np.float32),
            "mask": np.ascontiguousarray(inputs["mask"][lo:hi], dtype=np.int32),
            "W_h": np.asarray(inputs["W_h"], dtype=np.float32),
            "W_s": np.asarray(inputs["W_s"], dtype=np.float32),
            "v": np.asarray(inputs["v"], dtype=np.float32),
        })
    return in_maps


def kernel(decoder_hidden, encoder_outputs, mask, W_h, W_s, v):
    nc = _get_nc()
    in_maps = _make_in_maps(dict(
        decoder_hidden=decoder_hidden, encoder_outputs=encoder_outputs,
        mask=mask, W_h=W_h, W_s=W_s, v=v))
    res = run_bass_kernel_spmd(nc, in_maps, core_ids=list(range(NCORES)))
    context = np.concatenate(
        [res.results[i]["context"] for i in range(NCORES)], axis=0)
    attn = np.concatenate(
        [res.results[i]["attn_weights"] for i in range(NCORES)], axis=0)
    return (context.astype(np.float32), attn.astype(np.float32))
